# revision 9
# baseline (speedup 1.0000x reference)
"""Trainium2 Bass kernel for nn_Attention_28930899706081 (sparse_attention).

Reference computation:
  k1 = l2norm_c(Wqk @ fmap1), k2 = l2norm_c(Wqk @ fmap2), q = l2norm_c(Wqk @ dmap)
  sim_i = q^T k_i per batch  -> [b, n, n] with n = h*w = 4096
  attn_i = softmax(sim_i, axis=-1)[:, None]  -> [b, 1, n, n]
  returns (attn1, attn2)

Sharding: 8 cores; core i handles batch b = i//4 and query-row block r = i%4
(1024 of 4096 rows). Each core recomputes the full normalized K for its batch
and its row block of both sims + softmax.

ScalarE runs only the 32 softmax exps (+1 table-warm); rowsums ride the ACT
accumulator. Column inverse-norms: per-group ones-matmuls with interleaved
column groups give a compact [128, ng] layout, a bit-trick+Newton rsqrt runs
on VectorE, a gpsimd DMA flattens to [1, xch] (in order, thanks to the
interleaved grouping), and a K=1 ones-row matmul broadcasts it back across
partitions into the (by then free) proj PSUM tile for the column normalize.
Phase A is emitted software-pipelined in stages (P3 of chunk i-2, P2 of
chunk i-1, P1 of chunk i) so the PE queue never blocks on a cross-engine
round trip; PSUM is one shared 2x[128,2048] pool rotated between projection
chunks and sim tiles.
"""

import numpy as np
import ml_dtypes

B, C, H, W, D = 2, 256, 64, 64, 128
N = H * W  # 4096
QBLK = N // 4  # 1024 query rows per core
N_CORES = 8
CH = 2048   # sim/exp chunk (one PSUM tile)
PCH = 512   # matmul free-dim chunk (one PSUM bank)

_cached = {}


def _build():
    import concourse.mybir as mybir
    import concourse.tile as tile
    from concourse import bacc
    from contextlib import ExitStack

    f32 = mybir.dt.float32
    bf16 = mybir.dt.bfloat16
    i32 = mybir.dt.int32
    AF = mybir.ActivationFunctionType
    ALU = mybir.AluOpType

    nc = bacc.Bacc(
        "TRN2",
        target_bir_lowering=False,
        debug=False,
        enable_asserts=False,
        num_devices=N_CORES,
    )

    f1_ext = nc.dram_tensor("f1", [C, N], bf16, kind="ExternalInput").ap()
    f2_ext = nc.dram_tensor("f2", [C, N], bf16, kind="ExternalInput").ap()
    xq_ext = nc.dram_tensor("xq", [C, QBLK], bf16, kind="ExternalInput").ap()
    wqkT_ext = nc.dram_tensor("wqkT", [C, D], bf16, kind="ExternalInput").ap()
    out_ext = nc.dram_tensor("out", [2, QBLK, N], bf16, kind="ExternalOutput").ap()

    with tile.TileContext(nc) as tc, ExitStack() as ctx:
        consts = ctx.enter_context(tc.tile_pool(name="consts", bufs=1))
        xin = ctx.enter_context(tc.tile_pool(name="xin", bufs=12))
        ybf_p = ctx.enter_context(tc.tile_pool(name="ybf", bufs=3))
        ysq_p = ctx.enter_context(tc.tile_pool(name="ysq", bufs=3))
        nwt_p = ctx.enter_context(tc.tile_pool(name="nwt", bufs=3))
        flat_p = ctx.enter_context(tc.tile_pool(name="flat", bufs=3))
        kn_p = ctx.enter_context(tc.tile_pool(name="kn", bufs=1))
        e_p = ctx.enter_context(tc.tile_pool(name="epool", bufs=5))
        attn_p = ctx.enter_context(tc.tile_pool(name="attn", bufs=4))
        stat_p = ctx.enter_context(tc.tile_pool(name="stat", bufs=10))

        wqkT_sb = [
            consts.tile([128, D], bf16, tag=f"wqkT{k}", name=f"wqkT{k}")
            for k in range(2)
        ]
        nc.gpsimd.dma_start(out=wqkT_sb[0][:], in_=wqkT_ext[0:128, :])
        nc.gpsimd.dma_start(out=wqkT_sb[1][:], in_=wqkT_ext[128:256, :])
        ones_col = consts.tile([128, 1], bf16, tag="ones", name="ones")
        nc.vector.memset(ones_col[:], 1.0)
        ones_row = consts.tile([1, 128], bf16, tag="onesr", name="onesr")
        nc.vector.memset(ones_row[:], 1.0)
        # the only ACT table load of the whole kernel
        warm = consts.tile([128, 1], f32, tag="warm", name="warm")
        nc.scalar.activation(out=warm[:], in_=ones_col[:], func=AF.Exp)

        # -------- front-load all input DMAs (sync queue, priority order)
        def load_x(x_ext, chunks, tagbase):
            tiles = []
            for (h0, xch) in chunks:
                lo = xin.tile([128, CH], bf16, tag="xi", name=f"{tagbase}lo{h0}")
                hi = xin.tile([128, CH], bf16, tag="xi", name=f"{tagbase}hi{h0}")
                nc.sync.dma_start(out=lo[:, 0:xch], in_=x_ext[0:128, h0 : h0 + xch])
                nc.sync.dma_start(out=hi[:, 0:xch], in_=x_ext[128:256, h0 : h0 + xch])
                tiles.append((h0, xch, lo, hi))
            return tiles

        xq_t = load_x(xq_ext, [(0, 1024)], "xq")
        f1_t = load_x(f1_ext, [(0, 1024), (1024, 1024), (2048, 2048)], "f1")
        f2_t = load_x(f2_ext, [(0, 2048), (2048, 2048)], "f2")

        psum = ctx.enter_context(tc.tile_pool(name="P", bufs=2, space="PSUM"))

        # -------- phase A, staged --------
        def stage_p1(st):
            """proj matmuls + ysq + y_bf evac (PE visit 1 + DVE)."""
            kn, h0, xch, x_lo, x_hi = st["kn"], st["h0"], st["xch"], st["lo"], st["hi"]
            ps = psum.tile([128, CH], f32, tag="P", name=f"ps_{st['nm']}")
            st["ps"] = ps
            for c in range(xch // PCH):
                sl = slice(c * PCH, (c + 1) * PCH)
                nc.tensor.matmul(ps[:, sl], wqkT_sb[0][:], x_lo[:, sl],
                                 start=True, stop=False)
            for c in range(xch // PCH):
                sl = slice(c * PCH, (c + 1) * PCH)
                nc.tensor.matmul(ps[:, sl], wqkT_sb[1][:], x_hi[:, sl],
                                 start=False, stop=True)
            y_bf = ybf_p.tile([128, CH], bf16, tag="ybf", name=f"y_{st['nm']}")
            nc.vector.tensor_copy(y_bf[:, 0:xch], ps[:, 0:xch])
            ysq = ysq_p.tile([128, CH], bf16, tag="ysq", name=f"sq_{st['nm']}")
            nc.vector.tensor_mul(ysq[:, 0:xch], y_bf[:, 0:xch], y_bf[:, 0:xch])
            st["ysq"], st["ybf"] = ysq, y_bf

        def stage_p2(st):
            """compact col-norms + newton rsqrt + flatten (PE visit 2)."""
            ps, ysq, xch = st["ps"], st["ysq"], st["xch"]
            ng = xch // 128
            # group g = cols {m*ng + g}: n2_t[p, g] = |col p*ng+g|^2 so the
            # partition-major flatten is in order
            for g in range(ng):
                nc.tensor.matmul(ps[:, g : g + 1], ysq[:, g : xch : ng],
                                 ones_col[:], start=True, stop=True)
            nsl = slice(0, ng)
            n2c = nwt_p.tile([128, 16], f32, tag="n2c", name=f"n2_{st['nm']}")
            nc.vector.tensor_copy(n2c[:, nsl], ps[:, nsl])
            ish = nwt_p.tile([128, 16], i32, tag="ish", name=f"is_{st['nm']}")
            nc.vector.tensor_scalar(
                out=ish[:, nsl], in0=n2c[:, nsl].bitcast(i32), scalar1=1,
                scalar2=0xFFFFFFFF, op0=ALU.arith_shift_right,
                op1=ALU.bitwise_xor)
            seed = nwt_p.tile([128, 16], i32, tag="seed", name=f"sd_{st['nm']}")
            nc.vector.tensor_scalar_add(seed[:, nsl], ish[:, nsl], 0x5F3759E0)
            r0 = seed[:, nsl].bitcast(f32)
            t0 = nwt_p.tile([128, 16], f32, tag="t0", name=f"t0_{st['nm']}")
            rkt = nwt_p.tile([128, 16], bf16, tag="rkt", name=f"rk_{st['nm']}")
            # one fused newton iteration: rk = r0*(1.5 - 0.5*x*r0^2)
            nc.vector.tensor_mul(t0[:, nsl], r0, n2c[:, nsl])
            nc.vector.scalar_tensor_tensor(
                out=t0[:, nsl], in0=t0[:, nsl], scalar=-0.5, in1=r0,
                op0=ALU.mult, op1=ALU.mult)
            nc.vector.scalar_tensor_tensor(
                out=rkt[:, nsl], in0=t0[:, nsl], scalar=1.5, in1=r0,
                op0=ALU.add, op1=ALU.mult)
            flat = flat_p.tile([1, CH], bf16, tag="flat", name=f"fl_{st['nm']}")
            nc.gpsimd.dma_start(out=flat[0:1, 0:xch], in_=rkt[:, nsl])
            st["flat"] = flat

        def stage_p3(st):
            """rk partition-broadcast matmul + column normalize (PE visit 3)."""
            ps, flat, ybf, kn, h0, xch = (st["ps"], st["flat"], st["ybf"],
                                          st["kn"], st["h0"], st["xch"])
            for c in range(xch // PCH):
                sl = slice(c * PCH, (c + 1) * PCH)
                nc.tensor.matmul(ps[:, sl], ones_row[0:1, :], flat[0:1, sl],
                                 start=True, stop=True)
            nc.vector.tensor_mul(kn[:, h0 : h0 + xch], ybf[:, 0:xch],
                                 ps[:, 0:xch])

        qn = kn_p.tile([128, QBLK], bf16, tag="qn", name="qn")
        k1n = kn_p.tile([128, N], bf16, tag="k1n", name="k1n")
        k2n = kn_p.tile([128, N], bf16, tag="k2n", name="k2n")

        chunks = []
        for (h0, xch, lo, hi) in xq_t:
            chunks.append({"kn": qn, "h0": h0, "xch": xch, "lo": lo, "hi": hi,
                           "nm": f"q{h0}"})
        for (h0, xch, lo, hi) in f1_t:
            chunks.append({"kn": k1n, "h0": h0, "xch": xch, "lo": lo, "hi": hi,
                           "nm": f"k1_{h0}"})
        for (h0, xch, lo, hi) in f2_t:
            chunks.append({"kn": k2n, "h0": h0, "xch": xch, "lo": lo, "hi": hi,
                           "nm": f"k2_{h0}"})

        # software-pipelined emission: P3(i-2), P2(i-1), P1(i)
        for i in range(len(chunks)):
            if i >= 2:
                stage_p3(chunks[i - 2])
            if i >= 1:
                stage_p2(chunks[i - 1])
            stage_p1(chunks[i])
        stage_p3(chunks[-2])
        stage_p2(chunks[-1])
        stage_p3(chunks[-1])

        # -------- phase B --------
        def sim_chunk(kn, s, t, j, stile):
            lhsT = qn[:, t * 128 : (t + 1) * 128]
            ps = psum.tile([128, CH], f32, tag="P", name=f"sim{s}_{t}_{j}")
            for c in range(CH // PCH):
                csl = slice(j * CH + c * PCH, j * CH + (c + 1) * PCH)
                nc.tensor.matmul(ps[:, c * PCH : (c + 1) * PCH], lhsT,
                                 kn[:, csl], start=True, stop=True)
            e = e_p.tile([128, CH], bf16, tag="e", name=f"e{s}_{t}_{j}")
            nc.scalar.activation(out=e[:], in_=ps[:], func=AF.Exp,
                                 accum_out=stile[:, j : j + 1])
            return e

        def finish_tile(s, t, stile, e_chunks):
            ssum = stat_p.tile([128, 1], f32, tag="ssum", name=f"zs{s}_{t}")
            nc.vector.reduce_sum(ssum[:], stile[:], axis=mybir.AxisListType.X)
            recip = stat_p.tile([128, 1], f32, tag="recip", name=f"rc{s}_{t}")
            nc.vector.reciprocal(recip[:], ssum[:])
            for j, e in enumerate(e_chunks):
                attn = attn_p.tile([128, CH], bf16, tag="attn", name=f"a{s}_{t}_{j}")
                nc.vector.tensor_scalar_mul(attn[:], e[:], recip[:])
                nc.sync.dma_start(
                    out=out_ext[s, t * 128 : (t + 1) * 128,
                                j * CH : (j + 1) * CH],
                    in_=attn[:],
                )

        for s, kn in ((0, k1n), (1, k2n)):
            for t in range(QBLK // 128):
                stile = stat_p.tile([128, 2], f32, tag="stile", name=f"st{s}_{t}")
                e0 = sim_chunk(kn, s, t, 0, stile)
                e1 = sim_chunk(kn, s, t, 1, stile)
                finish_tile(s, t, stile, [e0, e1])

    nc.compile()
    return nc


def _get_nc():
    if "nc" not in _cached:
        _cached["nc"] = _build()
    return _cached["nc"]


def _in_maps(fmap1, fmap2, dmap, Wqk):
    bf = ml_dtypes.bfloat16
    f1r = np.asarray(fmap1, dtype=np.float32).reshape(B, C, N)
    f2r = np.asarray(fmap2, dtype=np.float32).reshape(B, C, N)
    dqr = np.asarray(dmap, dtype=np.float32).reshape(B, C, N)
    wT = np.ascontiguousarray(np.asarray(Wqk, dtype=np.float32).T).astype(bf)

    in_maps = []
    for i in range(N_CORES):
        b, r = divmod(i, 4)
        in_maps.append(
            {
                "f1": np.ascontiguousarray(f1r[b]).astype(bf),
                "f2": np.ascontiguousarray(f2r[b]).astype(bf),
                "xq": np.ascontiguousarray(
                    dqr[b][:, r * QBLK : (r + 1) * QBLK]
                ).astype(bf),
                "wqkT": wT,
            }
        )
    return in_maps


def kernel(fmap1, fmap2, dmap, Wqk):
    from concourse.bass_utils import run_bass_kernel_spmd

    in_maps = _in_maps(fmap1, fmap2, dmap, Wqk)
    nc = _get_nc()
    res = run_bass_kernel_spmd(nc, in_maps, core_ids=list(range(N_CORES)))
    _cached["last_result"] = res

    attn1 = np.empty((B, 1, N, N), dtype=np.float32)
    attn2 = np.empty((B, 1, N, N), dtype=np.float32)
    for i in range(N_CORES):
        b, r = divmod(i, 4)
        o = res.results[i]["out"]
        attn1[b, 0, r * QBLK : (r + 1) * QBLK, :] = o[0].astype(np.float32)
        attn2[b, 0, r * QBLK : (r + 1) * QBLK, :] = o[1].astype(np.float32)
    return (attn1, attn2)


# revision 12
# speedup vs baseline: 1.1692x; 1.1692x over previous
"""Trainium2 Bass kernel for nn_Attention_28930899706081 (sparse_attention).

Reference computation:
  k1 = l2norm_c(Wqk @ fmap1), k2 = l2norm_c(Wqk @ fmap2), q = l2norm_c(Wqk @ dmap)
  sim_i = q^T k_i per batch  -> [b, n, n] with n = h*w = 4096
  attn_i = softmax(sim_i, axis=-1)[:, None]  -> [b, 1, n, n]
  returns (attn1, attn2)

Sharding: 8 cores; core i handles batch b = i//4 and query-row block r = i%4
(1024 of 4096 rows). Each core computes the full normalized K for its batch
(recompute instead of collectives) and its row block of both sims + softmax.

Compute dtype bf16 (fp32 accumulation in PSUM); |sim| <= 1 because q/k are
unit vectors, so softmax needs no max subtraction. Row sums come from the
ScalarE activation accumulator fused with exp. Column L2 norms are computed
with a ones-matmul partition reduction (broadcast across partitions), and
1/sqrt comes from the single-pass Abs_reciprocal_sqrt activation (measured
~4e-5 rel err on HW). Output is written bf16 and upcast on the host.
"""

import numpy as np
import ml_dtypes

B, C, H, W, D = 2, 256, 64, 64, 128
N = H * W  # 4096
QBLK = N // 4  # 1024 query rows per core
N_CORES = 8

_cached = {}


def _build():
    import concourse.mybir as mybir
    import concourse.tile as tile
    from concourse.tile_rust import add_dep_helper
    from concourse import bacc
    from contextlib import ExitStack

    f32 = mybir.dt.float32
    f16 = mybir.dt.float16
    bf16 = mybir.dt.bfloat16
    AF = mybir.ActivationFunctionType

    nc = bacc.Bacc(
        "TRN2",
        target_bir_lowering=False,
        debug=False,
        enable_asserts=False,
        num_devices=N_CORES,
    )

    f1_ext = nc.dram_tensor("f1", [C, N], bf16, kind="ExternalInput").ap()
    f2_ext = nc.dram_tensor("f2", [C, N], bf16, kind="ExternalInput").ap()
    xq_ext = nc.dram_tensor("xq", [C, QBLK], bf16, kind="ExternalInput").ap()
    wqkT_ext = nc.dram_tensor("wqkT", [C, D], bf16, kind="ExternalInput").ap()
    out_ext = nc.dram_tensor("out", [2, QBLK, N], bf16, kind="ExternalOutput").ap()

    PCH = 512  # matmul free-dim chunk (one PSUM bank)
    CH = 2048  # pipeline chunk

    with tile.TileContext(nc) as tc, ExitStack() as ctx:
        consts = ctx.enter_context(tc.tile_pool(name="consts", bufs=1))
        xin = ctx.enter_context(tc.tile_pool(name="xin", bufs=20))
        ysq_pool = ctx.enter_context(tc.tile_pool(name="ysq", bufs=4))
        rk_pool = ctx.enter_context(tc.tile_pool(name="rk", bufs=4))
        kn_pool = ctx.enter_context(tc.tile_pool(name="kn", bufs=1))
        e_pool = ctx.enter_context(tc.tile_pool(name="epool", bufs=8))
        attn_pool = ctx.enter_context(tc.tile_pool(name="attn", bufs=4))
        stat_pool = ctx.enter_context(tc.tile_pool(name="stat", bufs=4))

        # constants
        wqkT_sb = [
            consts.tile([128, D], bf16, tag=f"wqkT{k}", name=f"wqkT{k}")
            for k in range(2)
        ]
        nc.gpsimd.dma_start(out=wqkT_sb[0][:], in_=wqkT_ext[0:128, :])
        nc.gpsimd.dma_start(out=wqkT_sb[1][:], in_=wqkT_ext[128:256, :])
        ones_sb = consts.tile([128, 128], bf16, tag="ones", name="ones")
        nc.vector.memset(ones_sb[:], 1.0)
        # prime the ACT table set: a dummy Abs_reciprocal_sqrt loads
        # abs_reciprocal_sqrt_and_small (which also contains Square), so
        # phase A runs on a single table load instead of square-set + ars-set
        warm = consts.tile([128, 1], f32, tag="warm", name="warm")
        nc.scalar.activation(out=warm[:], in_=ones_sb[:, 0:1], func=AF.Abs_reciprocal_sqrt)

        last_rk = None

        # front-load all input DMAs in consumption order
        def load_x(x_ext, ncols, xch, tagbase):
            tiles = []
            for h0 in range(0, ncols, xch):
                lo = xin.tile([128, xch], bf16, tag="xin", name=f"{tagbase}l{h0}")
                hi = xin.tile([128, xch], bf16, tag="xin", name=f"{tagbase}h{h0}")
                nc.sync.dma_start(out=lo[:], in_=x_ext[0:128, h0 : h0 + xch])
                nc.sync.dma_start(out=hi[:], in_=x_ext[128:256, h0 : h0 + xch])
                tiles.append((h0, xch, lo, hi))
            return tiles

        f1_t = load_x(f1_ext, N, 1024, "f1")
        f2_t = load_x(f2_ext, N, 1024, "f2")
        xq_t = load_x(xq_ext, QBLK, 512, "xq")

        k1n = kn_pool.tile([128, N], bf16, tag="k1n", name="k1n")
        k2n = kn_pool.tile([128, N], bf16, tag="k2n", name="k2n")
        qn = kn_pool.tile([128, QBLK], bf16, tag="qn", name="qn")

        chunks = []
        for (h0, xch, lo, hi) in f1_t:
            chunks.append({"xn": k1n, "h0": h0, "xch": xch, "lo": lo, "hi": hi})
        for (h0, xch, lo, hi) in f2_t:
            chunks.append({"xn": k2n, "h0": h0, "xch": xch, "lo": lo, "hi": hi})
        for (h0, xch, lo, hi) in xq_t:
            chunks.append({"xn": qn, "h0": h0, "xch": xch, "lo": lo, "hi": hi})

        with tc.tile_pool(name="proj_psum", bufs=2, space="PSUM") as proj_psum, \
             tc.tile_pool(name="n2_psum", bufs=2, space="PSUM") as n2_psum:

            def stage_proj(st):
                xch = st["xch"]
                ps = proj_psum.tile([128, 1024], f32, tag="proj", name="pps")
                st["ps"] = ps
                for c in range(xch // PCH):
                    sl = slice(c * PCH, (c + 1) * PCH)
                    nc.tensor.matmul(
                        ps[:, sl], wqkT_sb[0][:], st["lo"][:, sl],
                        start=True, stop=False)
                    nc.tensor.matmul(
                        ps[:, sl], wqkT_sb[1][:], st["hi"][:, sl],
                        start=False, stop=True)

            def stage_norm(st):
                xch = st["xch"]
                ps = st["ps"]
                ysq = ysq_pool.tile([128, 1024], bf16, tag="ysq", name="ysq")
                nc.scalar.activation(out=ysq[:, 0:xch], in_=ps[:, 0:xch],
                                     func=AF.Square)
                nps = n2_psum.tile([128, 1024], f32, tag="n2", name="nps")
                st["nps"] = nps
                for c in range(xch // PCH):
                    nc.tensor.matmul(
                        nps[:, c * PCH : (c + 1) * PCH],
                        ones_sb[:],
                        ysq[:, c * PCH : (c + 1) * PCH],
                        start=True,
                        stop=True,
                    )

            def stage_scale(st):
                nonlocal last_rk
                xch, h0 = st["xch"], st["h0"]
                rk = rk_pool.tile([128, 1024], f16, tag="rk", name="rk")
                last_rk = nc.scalar.activation(
                    out=rk[:, 0:xch], in_=st["nps"][:, 0:xch],
                    func=AF.Abs_reciprocal_sqrt)
                nc.vector.tensor_mul(st["xn"][:, h0 : h0 + xch],
                                     st["ps"][:, 0:xch], rk[:, 0:xch])

            # software-pipelined emission: scale(i-2), norm(i-1), proj(i)
            for i in range(len(chunks)):
                if i >= 2:
                    stage_scale(chunks[i - 2])
                if i >= 1:
                    stage_norm(chunks[i - 1])
                stage_proj(chunks[i])
            stage_scale(chunks[-2])
            stage_norm(chunks[-1])
            stage_scale(chunks[-1])

        with tc.tile_pool(name="sim_psum", bufs=2, space="PSUM") as sim_psum:
            first_exp = None

            def phase_b(kn, s):
                """row block of sim + softmax for one K map, streamed to out[s]."""
                nonlocal first_exp
                for t in range(QBLK // 128):
                    lhsT = qn[:, t * 128 : (t + 1) * 128]
                    attn = attn_pool.tile([128, N], bf16, tag="attn", name="attn")
                    stile = stat_pool.tile([128, 2], f32, tag="stile", name="stile")
                    e_chunks = []
                    for j in range(N // CH):
                        ps = sim_psum.tile([128, CH], f32, tag="sim", name="sim_ps")
                        for c in range(CH // PCH):
                            csl = slice(j * CH + c * PCH, j * CH + (c + 1) * PCH)
                            nc.tensor.matmul(
                                ps[:, c * PCH : (c + 1) * PCH],
                                lhsT,
                                kn[:, csl],
                                start=True,
                                stop=True,
                            )
                        e = e_pool.tile([128, CH], bf16, tag="e", name="e")
                        ex = nc.scalar.activation(
                            out=e[:],
                            in_=ps[:],
                            func=AF.Exp,
                            accum_out=stile[:, j : j + 1],
                        )
                        if first_exp is None:
                            first_exp = ex
                            # keep ACT table loads to 2: all Abs_reciprocal_sqrt
                            # (phase A) strictly before any Exp (phase B)
                            add_dep_helper(
                                ex.ins, last_rk.ins, sync=False,
                                reason="order rk (ars table) before exp table load",
                            )
                        e_chunks.append(e)
                    ssum = stat_pool.tile([128, 1], f32, tag="ssum", name="ssum")
                    nc.vector.reduce_sum(ssum[:], stile[:], axis=mybir.AxisListType.X)
                    recip = stat_pool.tile([128, 1], f32, tag="recip", name="recip")
                    nc.vector.reciprocal(recip[:], ssum[:])
                    for j, e in enumerate(e_chunks):
                        nc.vector.tensor_scalar_mul(
                            attn[:, j * CH : (j + 1) * CH], e[:], recip[:]
                        )
                        nc.sync.dma_start(
                            out=out_ext[
                                s, t * 128 : (t + 1) * 128, j * CH : (j + 1) * CH
                            ],
                            in_=attn[:, j * CH : (j + 1) * CH],
                        )

            phase_b(k1n, 0)
            phase_b(k2n, 1)

    nc.compile()
    return nc


def _get_nc():
    if "nc" not in _cached:
        _cached["nc"] = _build()
    return _cached["nc"]


def _in_maps(fmap1, fmap2, dmap, Wqk):
    bf = ml_dtypes.bfloat16
    f1r = np.asarray(fmap1, dtype=np.float32).reshape(B, C, N)
    f2r = np.asarray(fmap2, dtype=np.float32).reshape(B, C, N)
    dqr = np.asarray(dmap, dtype=np.float32).reshape(B, C, N)
    wT = np.ascontiguousarray(np.asarray(Wqk, dtype=np.float32).T).astype(bf)

    in_maps = []
    for i in range(N_CORES):
        b, r = divmod(i, 4)
        in_maps.append(
            {
                "f1": np.ascontiguousarray(f1r[b]).astype(bf),
                "f2": np.ascontiguousarray(f2r[b]).astype(bf),
                "xq": np.ascontiguousarray(
                    dqr[b][:, r * QBLK : (r + 1) * QBLK]
                ).astype(bf),
                "wqkT": wT,
            }
        )
    return in_maps


def kernel(fmap1, fmap2, dmap, Wqk):
    from concourse.bass_utils import run_bass_kernel_spmd

    in_maps = _in_maps(fmap1, fmap2, dmap, Wqk)
    nc = _get_nc()
    res = run_bass_kernel_spmd(nc, in_maps, core_ids=list(range(N_CORES)))
    _cached["last_result"] = res

    attn1 = np.empty((B, 1, N, N), dtype=np.float32)
    attn2 = np.empty((B, 1, N, N), dtype=np.float32)
    for i in range(N_CORES):
        b, r = divmod(i, 4)
        o = res.results[i]["out"]
        attn1[b, 0, r * QBLK : (r + 1) * QBLK, :] = o[0].astype(np.float32)
        attn2[b, 0, r * QBLK : (r + 1) * QBLK, :] = o[1].astype(np.float32)
    return (attn1, attn2)



# revision 13
# speedup vs baseline: 1.4503x; 1.2404x over previous
"""Trainium2 Bass kernel for nn_Attention_28930899706081 (sparse_attention).

Reference computation:
  k1 = l2norm_c(Wqk @ fmap1), k2 = l2norm_c(Wqk @ fmap2), q = l2norm_c(Wqk @ dmap)
  sim_i = q^T k_i per batch  -> [b, n, n] with n = h*w = 4096
  attn_i = softmax(sim_i, axis=-1)[:, None]  -> [b, 1, n, n]
  returns (attn1, attn2)

Sharding: 8 cores; core i handles batch b = i//4 and query-row block r = i%4
(1024 of 4096 rows). Each core computes the full normalized K for its batch
(recompute instead of collectives) and its row block of both sims + softmax.

Compute dtype bf16 (fp32 accumulation in PSUM); |sim| <= 1 because q/k are
unit vectors, so softmax needs no max subtraction. Row sums come from the
ScalarE activation accumulator fused with exp. Column L2 norms are computed
with a ones-matmul partition reduction (broadcast across partitions), and
1/sqrt comes from the single-pass Abs_reciprocal_sqrt activation (measured
~4e-5 rel err on HW). Output is written bf16 and upcast on the host.
"""

import numpy as np
import ml_dtypes

B, C, H, W, D = 2, 256, 64, 64, 128
N = H * W  # 4096
QBLK = N // 4  # 1024 query rows per core
N_CORES = 8

_cached = {}


def _build():
    import concourse.mybir as mybir
    import concourse.tile as tile
    from concourse.tile_rust import add_dep_helper
    from concourse import bacc
    from contextlib import ExitStack

    f32 = mybir.dt.float32
    f16 = mybir.dt.float16
    bf16 = mybir.dt.bfloat16
    AF = mybir.ActivationFunctionType

    nc = bacc.Bacc(
        "TRN2",
        target_bir_lowering=False,
        debug=False,
        enable_asserts=False,
        num_devices=N_CORES,
    )

    f1_ext = nc.dram_tensor("f1", [C, N], bf16, kind="ExternalInput").ap()
    f2_ext = nc.dram_tensor("f2", [C, N], bf16, kind="ExternalInput").ap()
    xq_ext = nc.dram_tensor("xq", [C, QBLK], bf16, kind="ExternalInput").ap()
    wqkT_ext = nc.dram_tensor("wqkT", [C, D], bf16, kind="ExternalInput").ap()
    out_ext = nc.dram_tensor("out", [2, QBLK, N], bf16, kind="ExternalOutput").ap()

    PCH = 512  # matmul free-dim chunk (one PSUM bank)
    CH = 2048  # pipeline chunk

    with tile.TileContext(nc) as tc, ExitStack() as ctx:
        consts = ctx.enter_context(tc.tile_pool(name="consts", bufs=1))
        xin = ctx.enter_context(tc.tile_pool(name="xin", bufs=12))
        ysq_pool = ctx.enter_context(tc.tile_pool(name="ysq", bufs=4))
        rk_pool = ctx.enter_context(tc.tile_pool(name="rk", bufs=4))
        kn_pool = ctx.enter_context(tc.tile_pool(name="kn", bufs=1))
        e_pool = ctx.enter_context(tc.tile_pool(name="epool", bufs=8))
        attn_pool = ctx.enter_context(tc.tile_pool(name="attn", bufs=4))
        stat_pool = ctx.enter_context(tc.tile_pool(name="stat", bufs=4))

        # constants
        wqkT_sb = [
            consts.tile([128, D], bf16, tag=f"wqkT{k}", name=f"wqkT{k}")
            for k in range(2)
        ]
        nc.gpsimd.dma_start(out=wqkT_sb[0][:], in_=wqkT_ext[0:128, :])
        nc.gpsimd.dma_start(out=wqkT_sb[1][:], in_=wqkT_ext[128:256, :])
        ones_sb = consts.tile([128, 128], bf16, tag="ones", name="ones")
        nc.vector.memset(ones_sb[:], 1.0)
        # prime the ACT table set: a dummy Abs_reciprocal_sqrt loads
        # abs_reciprocal_sqrt_and_small (which also contains Square), so
        # phase A runs on a single table load instead of square-set + ars-set
        warm = consts.tile([128, 1], f32, tag="warm", name="warm")
        nc.scalar.activation(out=warm[:], in_=ones_sb[:, 0:1], func=AF.Abs_reciprocal_sqrt)

        last_rk = None

        with tc.tile_pool(name="proj_psum", bufs=3, space="PSUM") as proj_psum, \
             tc.tile_pool(name="n2_psum", bufs=1, space="PSUM") as n2_psum:

            def phase_a(x_ext, ncols, tagbase, path):
                """DMA + project + l2-normalize columns, chunk-pipelined.

                path="act": y stays in PSUM; square on ScalarE, scale reads PSUM.
                path="dve": y evacuated to SBUF; square+scale on VectorE at 2x.
                The two paths let two maps' norm chains run on different
                engines concurrently. rk is fp16 so DVE muls stay 16-bit.
                """
                nonlocal last_rk
                xn = kn_pool.tile([128, ncols], bf16, tag=tagbase, name=tagbase)
                XCH = 512 if ncols <= 1024 else 1024
                for h in range(ncols // XCH):
                    h0 = h * XCH
                    x_lo = xin.tile([128, XCH], bf16, tag="xin", name="x_lo")
                    x_hi = xin.tile([128, XCH], bf16, tag="xin", name="x_hi")
                    nc.sync.dma_start(out=x_lo[:], in_=x_ext[0:128, h0 : h0 + XCH])
                    nc.sync.dma_start(out=x_hi[:], in_=x_ext[128:256, h0 : h0 + XCH])

                    ps = proj_psum.tile([128, XCH], f32, tag="proj", name="pps")
                    for c in range(XCH // PCH):
                        sl = slice(c * PCH, (c + 1) * PCH)
                        psl = ps[:, sl]
                        nc.tensor.matmul(
                            psl, wqkT_sb[0][:], x_lo[:, sl], start=True, stop=False
                        )
                        nc.tensor.matmul(
                            psl, wqkT_sb[1][:], x_hi[:, sl], start=False, stop=True
                        )

                    ysq = ysq_pool.tile([128, XCH], bf16, tag="ysq", name="ysq")
                    if path == "act":
                        nc.scalar.activation(out=ysq[:], in_=ps[:], func=AF.Square)
                    else:
                        y_bf = ysq_pool.tile([128, XCH], bf16, tag="ybf", name="y_bf")
                        nc.vector.tensor_copy(y_bf[:], ps[:])
                        nc.vector.tensor_mul(ysq[:], y_bf[:], y_bf[:])

                    nps = n2_psum.tile([128, XCH], f32, tag="n2", name="nps")
                    for c in range(XCH // PCH):
                        nc.tensor.matmul(
                            nps[:, c * PCH : (c + 1) * PCH],
                            ones_sb[:],
                            ysq[:, c * PCH : (c + 1) * PCH],
                            start=True,
                            stop=True,
                        )
                    # rk = n2^-0.5, already broadcast across partitions
                    rk = rk_pool.tile([128, XCH], f16, tag="rk", name="rk")
                    last_rk = nc.scalar.activation(
                        out=rk[:], in_=nps[:], func=AF.Abs_reciprocal_sqrt
                    )
                    if path == "act":
                        nc.vector.tensor_mul(xn[:, h0 : h0 + XCH], ps[:], rk[:])
                    else:
                        nc.vector.tensor_mul(xn[:, h0 : h0 + XCH], y_bf[:], rk[:])
                return xn

            k1n = phase_a(f1_ext, N, "k1n", "act")
            k2n = phase_a(f2_ext, N, "k2n", "act")
            qn = phase_a(xq_ext, QBLK, "qn", "act")

        with tc.tile_pool(name="sim_psum", bufs=2, space="PSUM") as sim_psum:
            first_exp = None

            def phase_b(kn, s):
                """row block of sim + softmax for one K map, streamed to out[s]."""
                nonlocal first_exp
                for t in range(QBLK // 128):
                    lhsT = qn[:, t * 128 : (t + 1) * 128]
                    attn = attn_pool.tile([128, N], bf16, tag="attn", name="attn")
                    stile = stat_pool.tile([128, 2], f32, tag="stile", name="stile")
                    e_chunks = []
                    for j in range(N // CH):
                        ps = sim_psum.tile([128, CH], f32, tag="sim", name="sim_ps")
                        for c in range(CH // PCH):
                            csl = slice(j * CH + c * PCH, j * CH + (c + 1) * PCH)
                            nc.tensor.matmul(
                                ps[:, c * PCH : (c + 1) * PCH],
                                lhsT,
                                kn[:, csl],
                                start=True,
                                stop=True,
                            )
                        e = e_pool.tile([128, CH], bf16, tag="e", name="e")
                        ex = nc.scalar.activation(
                            out=e[:],
                            in_=ps[:],
                            func=AF.Exp,
                            accum_out=stile[:, j : j + 1],
                        )
                        if first_exp is None:
                            first_exp = ex
                            # keep ACT table loads to 2: all Abs_reciprocal_sqrt
                            # (phase A) strictly before any Exp (phase B)
                            add_dep_helper(
                                ex.ins, last_rk.ins, sync=False,
                                reason="order rk (ars table) before exp table load",
                            )
                        e_chunks.append(e)
                    ssum = stat_pool.tile([128, 1], f32, tag="ssum", name="ssum")
                    nc.vector.reduce_sum(ssum[:], stile[:], axis=mybir.AxisListType.X)
                    recip = stat_pool.tile([128, 1], f32, tag="recip", name="recip")
                    nc.vector.reciprocal(recip[:], ssum[:])
                    for j, e in enumerate(e_chunks):
                        nc.vector.tensor_scalar_mul(
                            attn[:, j * CH : (j + 1) * CH], e[:], recip[:]
                        )
                        nc.sync.dma_start(
                            out=out_ext[
                                s, t * 128 : (t + 1) * 128, j * CH : (j + 1) * CH
                            ],
                            in_=attn[:, j * CH : (j + 1) * CH],
                        )

            phase_b(k1n, 0)
            phase_b(k2n, 1)

    nc.compile()
    return nc


def _get_nc():
    if "nc" not in _cached:
        _cached["nc"] = _build()
    return _cached["nc"]


def _in_maps(fmap1, fmap2, dmap, Wqk):
    bf = ml_dtypes.bfloat16
    f1r = np.asarray(fmap1, dtype=np.float32).reshape(B, C, N)
    f2r = np.asarray(fmap2, dtype=np.float32).reshape(B, C, N)
    dqr = np.asarray(dmap, dtype=np.float32).reshape(B, C, N)
    wT = np.ascontiguousarray(np.asarray(Wqk, dtype=np.float32).T).astype(bf)

    in_maps = []
    for i in range(N_CORES):
        b, r = divmod(i, 4)
        in_maps.append(
            {
                "f1": np.ascontiguousarray(f1r[b]).astype(bf),
                "f2": np.ascontiguousarray(f2r[b]).astype(bf),
                "xq": np.ascontiguousarray(
                    dqr[b][:, r * QBLK : (r + 1) * QBLK]
                ).astype(bf),
                "wqkT": wT,
            }
        )
    return in_maps


def kernel(fmap1, fmap2, dmap, Wqk):
    from concourse.bass_utils import run_bass_kernel_spmd

    in_maps = _in_maps(fmap1, fmap2, dmap, Wqk)
    nc = _get_nc()
    res = run_bass_kernel_spmd(nc, in_maps, core_ids=list(range(N_CORES)))
    _cached["last_result"] = res

    attn1 = np.empty((B, 1, N, N), dtype=np.float32)
    attn2 = np.empty((B, 1, N, N), dtype=np.float32)
    for i in range(N_CORES):
        b, r = divmod(i, 4)
        o = res.results[i]["out"]
        attn1[b, 0, r * QBLK : (r + 1) * QBLK, :] = o[0].astype(np.float32)
        attn2[b, 0, r * QBLK : (r + 1) * QBLK, :] = o[1].astype(np.float32)
    return (attn1, attn2)

